# revision 1
# baseline (speedup 1.0000x reference)
"""Trainium2 kernel for nn_DoubleAffineNet.

Math: the module's output is phi + psi - I where phi, psi are 3x3 affine
matrices built from pooled image statistics. phi needs mean(x), mean(y).
psi needs mean(x) and mean(y_comp), where y_comp is y bilinearly warped by
the near-identity affine map phi^{-1}.

Key identity: only the MEAN of y_comp is needed. Writing the warp-mean as
sum_{p,q} Y[p,q] * G[p,q] (G = bilinear splat weights of the affinely
mapped output lattice), a partition-of-unity argument shows that for
sub-pixel displacement fields (|u|,|v| < 0.5, which holds for this
problem's near-identity maps; asserted at runtime on the host), G is the
constant kappa = (1-a')(1-d') + b*c everywhere except the four border
rows/cols. Hence

    sum(y_comp) = kappa * sum(y) + sum_border Y*(G_true - kappa)

The device kernel therefore only computes the memory-bound statistics:
per-sample sum(x), sum(y), and the four border strips of y. The remaining
O(B*(3x3 + 4*1024)) algebra runs on the host (f32 where the reference is
f32, f64 for the border correction).

Sharding: pure data parallel, one sample per NeuronCore (B=8, 8 cores).

Device program (raw bacc, no TileContext — avoids its expensive
end-of-kernel drain/barrier):
  - 9 input DMAs: 4x + 3y of [128, 2048] (chunk c holds rows
    c*256 + {p, 128+p}) plus 2 half-size y tail chunks [128, 1024]
    (smaller tail chunks cut the end-of-stream reduce latency)
  - Vector reduces the x chunks + the last y chunk; Scalar reduces the
    other y chunks via ACT accum_out; GpSimd copies the border-column
    strips; the row strips go straight from SBUF to DRAM as soon as
    their own chunk lands (per-chunk semaphores), overlapped with the
    stream
  - everything lands in one packed [128, 26] "smalls" tile + 2 row strips
  - host does the final ~KB of reduction/algebra in float64
"""

import numpy as np

H = 1024
W = 1024
OUT_LEN = 5376

_CACHE = {}


def _build_program():
    import contextlib

    import concourse.bacc as bacc
    from concourse import mybir

    f32 = mybir.dt.float32
    Copy = mybir.ActivationFunctionType.Copy
    nc = bacc.Bacc("TRN2", target_bir_lowering=False, debug=False, num_devices=8)

    xd = nc.dram_tensor("x", [H, W], f32, kind="ExternalInput").ap()
    yd = nc.dram_tensor("y", [H, W], f32, kind="ExternalInput").ap()
    outd = nc.dram_tensor("out", [OUT_LEN], f32, kind="ExternalOutput").ap()

    with contextlib.ExitStack() as ctx:
        # chunks 0..3: x rows c*256+{p,128+p} as [128, 2048]
        # chunks 4..6: y rows likewise; chunks 7, 8: y rows 768..895 /
        # 896..1023 as [128, 1024] (small tail chunks cut the final latency)
        bufs = [
            ctx.enter_context(
                nc.sbuf_tensor(f"chunk{k}", [128, 2 * W if k < 7 else W], f32)
            )
            for k in range(9)
        ]
        # smalls cols: 0..3 x-partials, 4..8 y-partials (y0,y1,y2,y3a,y3b;
        # col 9 unused), 10..17 col0 strip, 18..25 col1023 strip
        # (strip col index = row//128)
        smalls = ctx.enter_context(nc.sbuf_tensor("smalls", [128, 26], f32))
        scratch = ctx.enter_context(nc.sbuf_tensor("scratch", [128, 2 * W], f32))
        # one semaphore per input chunk: wait_ge(sem_k, 16) proves ALL of
        # chunk k's queue segments landed. Cumulative counts on one sem can
        # release early under queue imbalance (a straggler queue may still
        # hold chunk-k bytes while later chunks' segments raise the total).
        dma_in_k = [
            ctx.enter_context(nc.semaphore(f"dma_in{k}")) for k in range(9)
        ]
        done = ctx.enter_context(nc.semaphore("done"))
        dma_out = ctx.enter_context(nc.semaphore("dma_out"))
        block = ctx.enter_context(nc.Block(no_gpsimd_drain=True))

        def src_chunk(k):
            if k < 4:
                return xd[(k % 4) * 256:(k % 4 + 1) * 256, :].rearrange(
                    "(a p) q -> p a q", a=2
                )
            if k < 7:
                c = k - 4
                return yd[c * 256:(c + 1) * 256, :].rearrange(
                    "(a p) q -> p a q", a=2
                )
            c = k - 7
            return yd[768 + c * 128 : 768 + (c + 1) * 128, :]

        @block.sync
        def _(sync):
            for k in range(9):
                dst = (
                    bufs[k].ap().rearrange("p (a q) -> p a q", a=2)
                    if k < 7
                    else bufs[k][:]
                )
                sync.dma_start(out=dst, in_=src_chunk(k)).then_inc(dma_in_k[k], 16)
            # row strips straight from the resident y chunks; issued while
            # the engines finish the last chunk, so they overlap the tail
            sync.wait_ge(dma_in_k[4], 16)
            sync.dma_start(
                out=outd[3328:4352].rearrange("(p q) -> p q", p=1),
                in_=bufs[4][0:1, 0:W],
            ).then_inc(dma_out, 16)
            sync.wait_ge(dma_in_k[8], 16)
            sync.dma_start(
                out=outd[4352:5376].rearrange("(p q) -> p q", p=1),
                in_=bufs[8][127:128, :],
            ).then_inc(dma_out, 16)
            sync.wait_ge(done, 13)
            sync.dma_start(
                out=outd[0:3328].rearrange("(p c) -> p c", c=26),
                in_=smalls[:],
            ).then_inc(dma_out, 16)
            sync.wait_ge(dma_out, 48)

        @block.vector
        def _(vector):
            # x chunks
            for k in range(4):
                vector.wait_ge(dma_in_k[k], 16)
                nc.vector.tensor_reduce(
                    out=smalls[:, k : k + 1],
                    in_=bufs[k][:],
                    axis=mybir.AxisListType.X,
                    op=mybir.AluOpType.add,
                ).then_inc(done, 1)
            # the whole last y chunk: scalar is still reducing y3a when
            # this chunk lands, so DVE finishes sooner than a split would
            vector.wait_ge(dma_in_k[8], 16)
            nc.vector.tensor_reduce(
                out=smalls[:, 8:9],
                in_=bufs[8][:],
                axis=mybir.AxisListType.X,
                op=mybir.AluOpType.add,
            ).then_inc(done, 1)

        @block.scalar
        def _(scalar):
            # y reduces via ACT accumulate (keeps DVE free for x)
            for j in range(3):
                k = 4 + j
                scalar.wait_ge(dma_in_k[k], 16)
                nc.scalar.activation(
                    scratch[:], bufs[k][:], Copy,
                    accum_out=smalls[:, 4 + j : 5 + j],
                ).then_inc(done, 1)
            scalar.wait_ge(dma_in_k[7], 16)
            nc.scalar.activation(
                scratch[:, 0:W], bufs[7][:], Copy,
                accum_out=smalls[:, 7:8],
            ).then_inc(done, 1)


        @block.gpsimd
        def _(gpsimd):
            # border-column strips
            for j in range(3):
                k = 4 + j
                gpsimd.wait_ge(dma_in_k[k], 16)
                t3 = bufs[k].ap().rearrange("p (a q) -> p a q", a=2)
                nc.gpsimd.tensor_copy(smalls[:, 10 + 2 * j : 12 + 2 * j], t3[:, :, 0])
                nc.gpsimd.tensor_copy(
                    smalls[:, 18 + 2 * j : 20 + 2 * j], t3[:, :, W - 1]
                ).then_inc(done, 1)
            gpsimd.wait_ge(dma_in_k[7], 16)
            nc.gpsimd.tensor_copy(smalls[:, 16:17], bufs[7][:, 0:1])
            nc.gpsimd.tensor_copy(smalls[:, 24:25], bufs[7][:, W - 1 : W])
            gpsimd.wait_ge(dma_in_k[8], 16)
            nc.gpsimd.tensor_copy(smalls[:, 17:18], bufs[8][:, 0:1])
            nc.gpsimd.tensor_copy(
                smalls[:, 25:26], bufs[8][:, W - 1 : W]
            ).then_inc(done, 1)

    nc.compile()
    return nc


def _get_program():
    if "nc" not in _CACHE:
        _CACHE["nc"] = _build_program()
    return _CACHE["nc"]


def _tent(z):
    return np.maximum(0.0, 1.0 - np.abs(z))


def _warp_mean_exact(y_img, A):
    """Fallback: honest bilinear warp-mean in numpy (used only if the
    sub-pixel displacement assumption fails, which it does not for this
    problem's inputs)."""
    A64 = A.astype(np.float64)
    i = np.arange(H, dtype=np.float64)[:, None]
    j = np.arange(W, dtype=np.float64)[None, :]
    px = A64[0, 0] * i + A64[0, 1] * j + 1023.0 * A64[0, 2]
    py = A64[1, 0] * i + A64[1, 1] * j + 1023.0 * A64[1, 2]
    x0 = np.floor(px).astype(np.int64)
    y0 = np.floor(py).astype(np.int64)
    wx = px - x0
    wy = py - y0
    im = y_img.astype(np.float64)
    acc = np.zeros((H, W))
    for xi, yi, w in (
        (x0, y0, (1 - wx) * (1 - wy)),
        (x0, y0 + 1, (1 - wx) * wy),
        (x0 + 1, y0, wx * (1 - wy)),
        (x0 + 1, y0 + 1, wx * wy),
    ):
        valid = (xi >= 0) & (xi < H) & (yi >= 0) & (yi < W)
        acc += im[np.clip(xi, 0, H - 1), np.clip(yi, 0, W - 1)] * w * valid
    return acc.mean()


def _warp_sum(sum_y, row0, row1, c0, c1, A):
    """sum(y_comp) from sum(y) + border strips, given phi_inv = A (f32).

    Requires the sub-pixel displacement assumption |u|,|v| < 0.5 (checked
    at the field corners; the fields are affine so corners bound the
    interior). The caller falls back to _warp_mean_exact otherwise.
    """
    A64 = A.astype(np.float64)
    ap, bb = A64[0, 0] - 1.0, A64[0, 1]
    cc, dp = A64[1, 0], A64[1, 1] - 1.0
    e1, e2 = 1023.0 * A64[0, 2], 1023.0 * A64[1, 2]

    mu = max(abs(ap * i + bb * j + e1) for i in (0.0, 1023.0) for j in (0.0, 1023.0))
    mv = max(abs(cc * i + dp * j + e2) for i in (0.0, 1023.0) for j in (0.0, 1023.0))
    assert mu < 0.5 and mv < 0.5, (mu, mv)

    kappa = (1.0 - ap) * (1.0 - dp) + bb * cc

    def g_true(p, q):
        g = np.zeros(np.broadcast(p, q).shape)
        for di in (-1, 0, 1):
            for dj in (-1, 0, 1):
                i_, j_ = p - di, q - dj
                valid = (i_ >= 0) & (i_ < H) & (j_ >= 0) & (j_ < W)
                z1 = ap * i_ + bb * j_ + e1 - di
                z2 = cc * i_ + dp * j_ + e2 - dj
                g += _tent(z1) * _tent(z2) * valid
        return g

    qs = np.arange(W, dtype=np.float64)
    ps = np.arange(1, H - 1, dtype=np.float64)
    ds = 0.0
    ds += np.sum(row0.astype(np.float64) * (g_true(0.0, qs) - kappa))
    ds += np.sum(row1.astype(np.float64) * (g_true(1023.0, qs) - kappa))
    ds += np.sum(c0[1:-1].astype(np.float64) * (g_true(ps, 0.0) - kappa))
    ds += np.sum(c1[1:-1].astype(np.float64) * (g_true(ps, 1023.0) - kappa))

    return kappa * float(sum_y) + ds


def _affine_f32(feat32, Wl, bl):
    M = (feat32 @ Wl + bl).reshape(3, 3)
    return np.eye(3, dtype=np.float32) + np.float32(0.01) * M


def kernel(x, y, Wpsi, bpsi, Wphi, bphi):
    from concourse import bass_utils

    B = x.shape[0]
    assert x.shape == (B, 1, H, W) and y.shape == (B, 1, H, W)

    nc = _get_program()
    in_maps = [
        {"x": np.ascontiguousarray(x[b, 0]), "y": np.ascontiguousarray(y[b, 0])}
        for b in range(B)
    ]
    results = bass_utils.run_bass_kernel_spmd(
        nc, in_maps, core_ids=list(range(B))
    ).results

    out = np.empty((B, 3, 3), dtype=np.float32)
    inv_hw = 1.0 / float(H * W)
    for b in range(B):
        r = np.asarray(results[b]["out"], dtype=np.float32).reshape(-1)
        sm = r[0:3328].reshape(128, 26).astype(np.float64)
        sum_x = float(sm[:, 0:4].sum())
        sum_y = float(sm[:, 4:9].sum())
        # strip cols land p-major: sm[p, 10+kblk] = y[kblk*128 + p, col]
        c0 = sm[:, 10:18].T.ravel()
        c1 = sm[:, 18:26].T.ravel()
        row0 = r[3328:4352].astype(np.float64)
        row1 = r[4352:5376].astype(np.float64)

        mean_x = np.float32(sum_x * inv_hw)
        mean_y = np.float32(sum_y * inv_hw)
        phi = _affine_f32(np.array([mean_x, mean_y], np.float32), Wpsi, bpsi)
        A = np.linalg.inv(phi)

        try:
            mean_yc = np.float32(_warp_sum(sum_y, row0, row1, c0, c1, A) * inv_hw)
        except AssertionError:
            mean_yc = np.float32(_warp_mean_exact(y[b, 0], A))

        psi = _affine_f32(np.array([mean_x, mean_yc], np.float32), Wphi, bphi)
        out[b] = phi + psi - np.eye(3, dtype=np.float32)
    return out

